# revision 1
# baseline (speedup 1.0000x reference)
"""Multi-head causal self-attention with RoPE on 8 Trainium2 NeuronCores.

Problem: x[2,2048,2048], wq/wk/wv/wo[2048,2048] fp32, 16 heads (hd=128),
interleaved RoPE, causal softmax, Megatron-style tensor parallelism over
heads: 2 heads per core, wo row-sharded, partial outputs summed on host.

All matmuls run as float32r (fp32 rounded to 11-bit mantissa; ~1 cycle/row
warm when back-to-back). Host pre-rounds DRAM inputs to f32r; on-device
producers write f32r directly.

Layout strategy (per core, per batch b):
  - host supplies xT = x^T [d, s] (f32r) and weight slices pre-transposed
  - projections: qT,kT per head via lhsT=w-tile [d,e], rhs=xT [d,s512]
    -> q^T,k^T [e=128, s] directly; v natural [s, e] via lhsT=xT-subtile;
    RoPE fused right after each projection chunk:
    qrotT = RotL.T @ qT (signed pair-swap as a matmul), then
    q_roped = qT*cosT + qrotT*sinT on DVE (tables indexed [e, s])
  - attention per (b, j-block of 512 q), heads interleaved:
      scoresT[kv=128, q=512] = kT-tile.T @ qT-block   (one matmul, d=128)
      staircase tiles compute only valid columns [delta:512]
      attn = exp(scoresT) on ACT (-> f32r); triangle mask on 128-col band
      oT[d, q] += v-tile.T @ attn ; rowsumB[128, q] += ones.T @ attn
      oT_norm = oT * reciprocal_approx_fast(rowsumB)  (-> f32r)
  - output projection: yT[e, s] = sum_ct woT-tile.T @ oT  -> DRAM
  - host: y = sum over cores of yT^T
"""

import os
import sys

for _p in ("/opt/trn_rl_repo", "/root/.axon_site/_ro/trn_rl_repo"):
    if os.path.isdir(_p) and _p not in sys.path:
        sys.path.append(_p)

import numpy as np

import concourse.bacc as bacc
import concourse.mybir as mybir
import concourse.tile as tile
from concourse.alu_op_type import AluOpType
from concourse.bass_utils import run_bass_kernel_spmd

F32 = mybir.dt.float32
F32R = mybir.dt.float32r
BF16 = mybir.dt.bfloat16

B, S, D = 2, 2048, 2048
H, HD = 16, 128
NCORES = 8
HPC = H // NCORES            # heads per core = 2
CPC = HPC * HD               # channels per core = 256
P = 128
SC = 512                     # s-chunk for projections / q-block for attention
NSC = S // SC                # 4
NDT = D // P                 # 16 contraction tiles
NG = 2                       # x-tile DMA group: d-tiles per DMA
ROPE_THETA = 10000.0

Exp = mybir.ActivationFunctionType.Exp

last_exec_time_ns = None
_nc_cache = None


def _round_f32r(x):
    u = np.ascontiguousarray(x, dtype=np.float32).view(np.uint32)
    r = (u + np.uint32(0x7FF) + ((u >> np.uint32(12)) & np.uint32(1))) \
        & np.uint32(0xFFFFF000)
    return r.view(np.float32)


def _build_nc():
    nc = bacc.Bacc("TRN2", target_bir_lowering=False, debug=False)

    xT = nc.dram_tensor("xT", [B, D, S], F32R, kind="ExternalInput")
    wqkvT = nc.dram_tensor("wqkvT", [D, 6 * P], F32R, kind="ExternalInput")
    woT = nc.dram_tensor("woT", [CPC, D], F32R, kind="ExternalInput")
    cosT = nc.dram_tensor("cosT", [HD, S], F32, kind="ExternalInput")
    sinT = nc.dram_tensor("sinT", [HD, S], F32, kind="ExternalInput")
    rotL = nc.dram_tensor("rotL", [HD, HD], F32R, kind="ExternalInput")
    trimask = nc.dram_tensor("trimask", [P, P], BF16, kind="ExternalInput")
    ones = nc.dram_tensor("ones", [P, P], F32R, kind="ExternalInput")
    yT = nc.dram_tensor("yT", [B, D, S], F32, kind="ExternalOutput")

    xTr = xT.rearrange("b (o p) s -> b p o s", p=P)

    with tile.TileContext(nc) as tc:
        with tc.tile_pool(name="const", bufs=1) as constp, \
             tc.tile_pool(name="xp", bufs=9) as xp, \
             tc.tile_pool(name="qk", bufs=1) as qkp, \
             tc.tile_pool(name="vp", bufs=1) as vp, \
             tc.tile_pool(name="op", bufs=1) as op_, \
             tc.tile_pool(name="attn", bufs=4) as attnp, \
             tc.tile_pool(name="tmp", bufs=2) as tmpp, \
             tc.tile_pool(name="yt", bufs=2) as ytp, \
             tc.tile_pool(name="ps", bufs=4, space="PSUM") as psp, \
             tc.tile_pool(name="acc", bufs=4, space="PSUM") as accp:

            # ---- constants (wq split per d-tile so matmuls start early;
            #      the rest deferred until after the first x-chunk DMAs) ----
            wq_sb = constp.tile([P, NDT, 6 * P], F32R)
            wqr = wqkvT.rearrange("(o p) e -> p o e", p=P)
            for dt in range(NDT):
                nc.sync.dma_start(wq_sb[:, dt, :], wqr[:, dt, :])
            wo_sb = constp.tile([P, CPC // P, D], F32R)
            cos_sb = constp.tile([P, S], F32)
            sin_sb = constp.tile([P, S], F32)
            rot_sb = constp.tile([P, P], F32R)
            mask_sb = constp.tile([P, P], BF16)
            ones_sb = constp.tile([P, P], F32R)

            def load_rest_of_consts():
                nc.sync.dma_start(rot_sb[:], rotL[:])
                nc.sync.dma_start(cos_sb[:], cosT[:])
                nc.sync.dma_start(sin_sb[:], sinT[:])
                nc.sync.dma_start(mask_sb[:], trimask[:])
                nc.sync.dma_start(ones_sb[:], ones[:])
                nc.sync.dma_start(wo_sb[:], woT.rearrange("(o p) e -> p o e", p=P))

            for b in range(B):
                # ---- projections (+ fused RoPE) ----
                # qkT[e] for e in {q_h0, q_h1, k_h0, k_h1}: [128, S] transposed
                qkT = [qkp.tile([P, S], F32R, tag=f"qk{e}", name=f"qkT{e}")
                       for e in range(4)]
                # v natural [s_in=128, s_out=16, ch=256]
                v_sb = vp.tile([P, NDT, CPC], F32R, tag="v")
                for sc in range(NSC):
                    xts = []
                    for g in range(NDT // NG):
                        xt = xp.tile([P, NG, SC], F32R, tag="xt")
                        nc.gpsimd.dma_start(
                            xt[:], xTr[b, :, g * NG:(g + 1) * NG,
                                       sc * SC:(sc + 1) * SC])
                        xts.append(xt)
                    if b == 0 and sc == 0:
                        load_rest_of_consts()
                    for e in range(4):
                        pq = accp.tile([P, SC], F32, tag="acc")
                        for dt in range(NDT):
                            nc.tensor.matmul(pq[:],
                                             wq_sb[:, dt, e * P:(e + 1) * P],
                                             xts[dt // NG][:, dt % NG, :],
                                             start=(dt == 0), stop=(dt == NDT - 1))
                        sl = slice(sc * SC, (sc + 1) * SC)
                        nc.scalar.copy(qkT[e][:, sl], pq[:])
                        # RoPE for this chunk, overlapped with projections
                        pr = psp.tile([P, SC], F32, tag="ps")
                        nc.tensor.matmul(pr[:], rot_sb[:], qkT[e][:, sl],
                                         start=True, stop=True)
                        tmp = tmpp.tile([P, SC], F32, tag="ropetmp")
                        nc.vector.tensor_tensor(tmp[:], pr[:], sin_sb[:, sl],
                                                AluOpType.mult)
                        nc.vector.tensor_tensor(qkT[e][:, sl], qkT[e][:, sl],
                                                cos_sb[:, sl], AluOpType.mult)
                        nc.vector.tensor_tensor(qkT[e][:, sl], qkT[e][:, sl],
                                                tmp[:], AluOpType.add)
                    for ss in range(SC // P):
                        pv = accp.tile([P, SC], F32, tag="acc")
                        pvv = pv[:, :CPC]
                        for dt in range(NDT):
                            nc.tensor.matmul(pvv,
                                             xts[dt // NG][:, dt % NG,
                                                           ss * P:(ss + 1) * P],
                                             wq_sb[:, dt, 4 * P:6 * P],
                                             start=(dt == 0), stop=(dt == NDT - 1))
                        nc.scalar.copy(v_sb[:, sc * (SC // P) + ss, :], pvv)

                # ---- attention: j outer, heads interleaved ----
                oT = op_.tile([P, HPC, S], F32R, tag="o")
                for j in range(NSC):
                    jsl = slice(j * SC, (j + 1) * SC)
                    n_kv = (SC // P) * (j + 1)
                    for h in range(HPC):
                        qTh, kTh = qkT[h], qkT[2 + h]
                        po = accp.tile([P, SC], F32, tag="acc")
                        prs = accp.tile([P, SC], F32, tag="acc")
                        for t in range(n_kv):
                            dp = t - (SC // P) * j
                            dlt = max(dp, 0) * P  # first valid column
                            vsl = slice(j * SC + dlt, (j + 1) * SC)
                            pscore = psp.tile([P, SC], F32, tag="ps")
                            nc.tensor.matmul(pscore[:, dlt:],
                                             kTh[:, t * P:(t + 1) * P],
                                             qTh[:, vsl],
                                             start=True, stop=True)
                            attn = attnp.tile([P, SC], F32R, tag="attn")
                            nc.scalar.activation(attn[:, dlt:], pscore[:, dlt:],
                                                 Exp, bias=0.0, scale=1.0)
                            if dp >= 0:  # triangle mask on the 128-col band
                                nc.vector.tensor_tensor(
                                    attn[:, dlt:dlt + P], attn[:, dlt:dlt + P],
                                    mask_sb[:], AluOpType.mult)
                            nc.tensor.matmul(po[:, dlt:],
                                             v_sb[:, t, h * HD:(h + 1) * HD],
                                             attn[:, dlt:],
                                             start=(t == 0), stop=(t == n_kv - 1),
                                             skip_group_check=True)
                            nc.tensor.matmul(prs[:, dlt:], ones_sb[:],
                                             attn[:, dlt:],
                                             start=(t == 0), stop=(t == n_kv - 1),
                                             skip_group_check=True)
                        recip = tmpp.tile([P, SC], F32, tag="recip")
                        nc.vector.reciprocal_approx_fast(recip[:], prs[:])
                        nc.vector.tensor_tensor(oT[:, h, jsl], po[:], recip[:],
                                                AluOpType.mult)

                    # ---- output projection for this q-block:
                    #      yT[e, jsl] = sum_ct woT.T @ oT ----
                    for eh in range(NDT // 2):
                        yt = ytp.tile([P, 2, SC], F32, tag="yt")
                        for si in range(2):
                            et = eh * 2 + si
                            py = accp.tile([P, SC], F32, tag="acc")
                            for ct in range(HPC):
                                nc.tensor.matmul(
                                    py[:],
                                    wo_sb[:, ct, et * P:(et + 1) * P],
                                    oT[:, ct, jsl],
                                    start=(ct == 0), stop=(ct == HPC - 1))
                            nc.scalar.copy(yt[:, si, :], py[:])
                        nc.sync.dma_start(
                            yT[b, eh * 2 * P:(eh + 1) * 2 * P, jsl]
                            .rearrange("(n p) q -> p n q", p=P),
                            yt[:])
    nc.finalize()
    return nc


def _host_inputs(x, wq, wk, wv, wo):
    """Build per-core input maps (host-side shard + transform)."""
    scale = 1.0 / np.sqrt(np.float32(HD))

    xTr = _round_f32r(np.ascontiguousarray(x.transpose(0, 2, 1)))

    # RoPE tables in [e, s] layout (same for every head)
    inv_freq = 1.0 / (ROPE_THETA ** (np.arange(0, HD, 2, dtype=np.float64) / HD))
    ang = np.arange(S, dtype=np.float64)[None, :] * inv_freq[:, None]  # [64, S]
    cosT = np.repeat(np.cos(ang), 2, axis=0).astype(np.float32)  # [128, S]
    sinT = np.repeat(np.sin(ang), 2, axis=0).astype(np.float32)

    # signed pair-swap: qrot[2i] = -q[2i+1], qrot[2i+1] = q[2i]
    # matmul computes qrot[m, s] = sum_k rotL[k, m] q[k, s]
    rotL = np.zeros((HD, HD), dtype=np.float32)
    for i in range(HD // 2):
        rotL[2 * i + 1, 2 * i] = -1.0
        rotL[2 * i, 2 * i + 1] = 1.0

    import ml_dtypes
    r = np.arange(P)[:, None]
    c = np.arange(P)[None, :]
    trimask = (c >= r).astype(ml_dtypes.bfloat16)  # [128,128] upper-right valid

    wq_s = _round_f32r(wq * scale)
    wk_s = _round_f32r(wk)
    wv_s = _round_f32r(wv)
    wo_s = _round_f32r(wo)

    in_maps = []
    for cix in range(NCORES):
        rows = slice(cix * CPC, (cix + 1) * CPC)  # head-channel rows
        blocks = []
        for h in range(HPC):
            hr = slice((cix * HPC + h) * HD, (cix * HPC + h + 1) * HD)
            blocks.append(wq_s[hr])   # q_h: [128, D]
        for h in range(HPC):
            hr = slice((cix * HPC + h) * HD, (cix * HPC + h + 1) * HD)
            blocks.append(wk_s[hr])
        blocks.append(wv_s[rows])     # v both heads: [256, D]
        wqkvT = np.ascontiguousarray(
            np.concatenate(blocks, axis=0).T)  # [D, 768]
        woT = np.ascontiguousarray(wo_s[:, rows].T)  # [256, D]
        in_maps.append({
            "xT": xTr,
            "wqkvT": wqkvT,
            "woT": woT,
            "cosT": cosT,
            "sinT": sinT,
            "rotL": rotL,
            "trimask": trimask,
            "ones": np.ones((P, P), dtype=np.float32),
        })
    return in_maps


def _get_nc():
    global _nc_cache
    if _nc_cache is None:
        _nc_cache = _build_nc()
    return _nc_cache


def kernel(x, wq, wk, wv, wo, _trace=False):
    global last_exec_time_ns
    nc = _get_nc()
    in_maps = _host_inputs(np.asarray(x, dtype=np.float32),
                           np.asarray(wq, dtype=np.float32),
                           np.asarray(wk, dtype=np.float32),
                           np.asarray(wv, dtype=np.float32),
                           np.asarray(wo, dtype=np.float32))
    res = run_bass_kernel_spmd(nc, in_maps, core_ids=list(range(NCORES)),
                               trace=_trace)
    last_exec_time_ns = res.exec_time_ns
    y = np.zeros((B, S, D), dtype=np.float64)
    for cix in range(NCORES):
        y += res.results[cix]["yT"].transpose(0, 2, 1).astype(np.float64)
    return y.astype(np.float32)



# revision 3
# speedup vs baseline: 1.1366x; 1.1366x over previous
"""Multi-head causal self-attention with RoPE on 8 Trainium2 NeuronCores.

Problem: x[2,2048,2048], wq/wk/wv/wo[2048,2048] fp32, 16 heads (hd=128),
interleaved RoPE, causal softmax.

Sharding v2: (batch, head-group) parallel — each core owns ONE batch and
FOUR heads (cores 0-3 -> b=0 head-groups 0-3, cores 4-7 -> b=1).  Halves
per-core x DMA vs pure head-parallel and removes the batch phase boundary.
wo is row-sharded; host sums the 4 partial y's per batch.

All data bf16 (same PE rate as f32r, half DMA/SBUF, FWL weight loads);
PSUM accumulation stays fp32.

Per core:
  - projections: qT,kT per head via lhsT=w-tile [d,e], rhs=xT [d,s512]
    -> [e=128, s] transposed; RoPE fused after each chunk (pair-swap as a
    small matmul + cos/sin DVE ops); v natural [s, ch=512].
    PSUM->SBUF evacuation on VectorE (ScalarE reserved for exp).
  - attention per (j-block of 512 q), heads processed in 2 pairs, kv tiles
    of 128 software-pipelined: scores(t) for both heads -> one exp call
    (3D AP covers both heads' valid staircase regions) -> attnV/rowsum
    matmuls of t-1 run while exp(t) is on ACT.
  - output projection per j-block: y[e, q] = sum_h woT-tile.T @ oT_norm.
"""

import os
import sys

for _p in ("/opt/trn_rl_repo", "/root/.axon_site/_ro/trn_rl_repo"):
    if os.path.isdir(_p) and _p not in sys.path:
        sys.path.append(_p)

import numpy as np

import concourse.bacc as bacc
import concourse.mybir as mybir
import concourse.tile as tile
from concourse.alu_op_type import AluOpType
from concourse.bass_utils import run_bass_kernel_spmd

F32 = mybir.dt.float32
BF16 = mybir.dt.bfloat16

B, S, D = 2, 2048, 2048
H, HD = 16, 128
NCORES = 8
HPC = 4                      # heads per core
CPC = HPC * HD               # channels per core = 512
P = 128
SC = 512                     # s-chunk for projections / q-block for attention
NSC = S // SC                # 4
NDT = D // P                 # 16 contraction tiles
NG = 4                       # x-tile DMA group: d-tiles per DMA
WCOLS = 8 * P + CPC          # 1536: q h0..3, k h0..3, v (512)
ROPE_THETA = 10000.0

Exp = mybir.ActivationFunctionType.Exp

last_exec_time_ns = None
_nc_cache = None


def _build_nc():
    nc = bacc.Bacc("TRN2", target_bir_lowering=False, debug=False)

    xT = nc.dram_tensor("xT", [D, S], BF16, kind="ExternalInput")
    wqkvT = nc.dram_tensor("wqkvT", [D, WCOLS], BF16, kind="ExternalInput")
    woT = nc.dram_tensor("woT", [CPC, D], BF16, kind="ExternalInput")
    cosT = nc.dram_tensor("cosT", [HD, S], BF16, kind="ExternalInput")
    sinT = nc.dram_tensor("sinT", [HD, S], BF16, kind="ExternalInput")
    rotL = nc.dram_tensor("rotL", [HD, HD], BF16, kind="ExternalInput")
    trimask = nc.dram_tensor("trimask", [P, 2 * P], BF16, kind="ExternalInput")
    ones = nc.dram_tensor("ones", [P, P], BF16, kind="ExternalInput")
    yT = nc.dram_tensor("yT", [D, S], BF16, kind="ExternalOutput")

    xTr = xT.rearrange("(o p) s -> p o s", p=P)

    with tile.TileContext(nc) as tc:
        with tc.tile_pool(name="const", bufs=1) as constp, \
             tc.tile_pool(name="xp", bufs=6) as xp, \
             tc.tile_pool(name="qk", bufs=1) as qkp, \
             tc.tile_pool(name="vp", bufs=1) as vp, \
             tc.tile_pool(name="op", bufs=1) as op_, \
             tc.tile_pool(name="attn", bufs=4) as attnp, \
             tc.tile_pool(name="tmp", bufs=2) as tmpp, \
             tc.tile_pool(name="rc", bufs=2) as rcp, \
             tc.tile_pool(name="yt", bufs=2) as ytp, \
             tc.tile_pool(name="ps", bufs=2, space="PSUM") as psp, \
             tc.tile_pool(name="acc", bufs=4, space="PSUM") as accp:

            # ---- constants (wq split per d-tile so matmuls start early) ----
            wq_sb = constp.tile([P, NDT, WCOLS], BF16)
            wqr = wqkvT.rearrange("(o p) e -> p o e", p=P)
            for dt in range(NDT):
                nc.sync.dma_start(wq_sb[:, dt, :], wqr[:, dt, :])
            wo_sb = constp.tile([P, HPC, D], BF16)
            cos_sb = constp.tile([P, S], BF16)
            sin_sb = constp.tile([P, S], BF16)
            rot_sb = constp.tile([P, P], BF16)
            mask_sb = constp.tile([P, 2, P], BF16)
            ones_sb = constp.tile([P, P], BF16)

            def load_rest_of_consts():
                nc.sync.dma_start(rot_sb[:], rotL[:])
                nc.sync.dma_start(cos_sb[:], cosT[:])
                nc.sync.dma_start(sin_sb[:], sinT[:])
                nc.sync.dma_start(mask_sb[:], trimask.rearrange("p (n q) -> p n q", n=2))
                nc.sync.dma_start(ones_sb[:], ones[:])
                nc.sync.dma_start(wo_sb[:], woT.rearrange("(o p) e -> p o e", p=P))

            # ---- projections (+ fused RoPE) ----
            # qkT[e] for e in {q_h0..3, k_h0..3}: [128, S] transposed
            qkT = [qkp.tile([P, S], BF16, tag=f"qk{e}", name=f"qkT{e}")
                   for e in range(8)]
            # v natural [s_in=128, s_out=16, ch=512]
            v_sb = vp.tile([P, NDT, CPC], BF16, tag="v")
            for sc in range(NSC):
                xts = []
                for g in range(NDT // NG):
                    xt = xp.tile([P, NG, SC], BF16, tag="xt")
                    nc.gpsimd.dma_start(
                        xt[:], xTr[:, g * NG:(g + 1) * NG,
                                   sc * SC:(sc + 1) * SC])
                    xts.append(xt)
                if sc == 0:
                    load_rest_of_consts()
                sl = slice(sc * SC, (sc + 1) * SC)
                for e in range(8):
                    pq = accp.tile([P, SC], F32, tag="acc")
                    for dt in range(NDT):
                        nc.tensor.matmul(pq[:],
                                         wq_sb[:, dt, e * P:(e + 1) * P],
                                         xts[dt // NG][:, dt % NG, :],
                                         start=(dt == 0), stop=(dt == NDT - 1))
                    # copy raw q to SBUF (needed for the rotation matmul)
                    nc.vector.tensor_scalar_mul(qkT[e][:, sl], pq[:], 1.0)
                    # RoPE: qrot = RotL.T @ q (signed pair swap), then
                    # q = q*cos + qrot*sin
                    pr = psp.tile([P, 2, SC], F32, tag="ps")
                    nc.tensor.matmul(pr[:, 0, :], rot_sb[:], qkT[e][:, sl],
                                     start=True, stop=True)
                    tmp = tmpp.tile([P, SC], BF16, tag="ropetmp")
                    nc.vector.tensor_tensor(tmp[:], pr[:, 0, :], sin_sb[:, sl],
                                            AluOpType.mult)
                    nc.vector.tensor_tensor(qkT[e][:, sl], qkT[e][:, sl],
                                            cos_sb[:, sl], AluOpType.mult)
                    nc.vector.tensor_tensor(qkT[e][:, sl], qkT[e][:, sl],
                                            tmp[:], AluOpType.add)
                for ss in range(SC // P):
                    pv = accp.tile([P, SC], F32, tag="acc")
                    for dt in range(NDT):
                        nc.tensor.matmul(pv[:],
                                         xts[dt // NG][:, dt % NG,
                                                       ss * P:(ss + 1) * P],
                                         wq_sb[:, dt, 8 * P:],
                                         start=(dt == 0), stop=(dt == NDT - 1))
                    nc.vector.tensor_scalar_mul(
                        v_sb[:, sc * (SC // P) + ss, :], pv[:], 1.0)

            # ---- attention: j outer, heads in 2 pairs, t software-pipelined ----
            oT = op_.tile([P, HPC, S], BF16, tag="o")
            for j in range(NSC):
                jsl = slice(j * SC, (j + 1) * SC)
                n_kv = (SC // P) * (j + 1)
                for pair in range(2):
                    hA, hB = 2 * pair, 2 * pair + 1
                    po = [accp.tile([P, SC], F32, tag="acc",
                                    name=f"po{j}_{pair}_{i}") for i in range(2)]
                    prs = [accp.tile([P, SC], F32, tag="acc",
                                     name=f"prs{j}_{pair}_{i}") for i in range(2)]
                    pend = []  # (attn_tile, t, dlt) awaiting attnV/rowsum

                    def drain_one():
                        attn, t, dlt = pend.pop(0)
                        first, last = (t == 0), (t == n_kv - 1)
                        for i, h in enumerate((hA, hB)):
                            nc.tensor.matmul(po[i][:, dlt:],
                                             v_sb[:, t, h * HD:(h + 1) * HD],
                                             attn[:, i, dlt:],
                                             start=first, stop=last,
                                             skip_group_check=True)
                            nc.tensor.matmul(prs[i][:, dlt:], ones_sb[:],
                                             attn[:, i, dlt:],
                                             start=first, stop=last,
                                             skip_group_check=True)

                    for t in range(n_kv):
                        dp = t - (SC // P) * j
                        dlt = max(dp, 0) * P  # first valid column
                        vsl = slice(j * SC + dlt, (j + 1) * SC)
                        tsl = slice(t * P, (t + 1) * P)
                        psc = psp.tile([P, 2, SC], F32, tag="ps")
                        for i, h in enumerate((hA, hB)):
                            nc.tensor.matmul(psc[:, i, dlt:],
                                             qkT[4 + h][:, tsl],
                                             qkT[h][:, vsl],
                                             start=True, stop=True)
                        attn = attnp.tile([P, 2, SC], BF16, tag="attn")
                        nc.scalar.activation(attn[:, :, dlt:], psc[:, :, dlt:],
                                             Exp, bias=0.0, scale=1.0)
                        if dp >= 0:  # triangle mask on the 128-col band
                            nc.vector.tensor_tensor(
                                attn[:, :, dlt:dlt + P], attn[:, :, dlt:dlt + P],
                                mask_sb[:], AluOpType.mult)
                        pend.append((attn, t, dlt))
                        if len(pend) > 1:
                            drain_one()
                    while pend:
                        drain_one()
                    for i, h in enumerate((hA, hB)):
                        recip = rcp.tile([P, SC], F32, tag="recip")
                        nc.vector.reciprocal_approx_fast(recip[:], prs[i][:])
                        nc.vector.tensor_tensor(oT[:, h, jsl], po[i][:],
                                                recip[:], AluOpType.mult)

                # ---- output projection for this q-block ----
                for eh in range(NDT // 2):
                    yt = ytp.tile([P, 2, SC], BF16, tag="yt")
                    for si in range(2):
                        et = eh * 2 + si
                        py = accp.tile([P, SC], F32, tag="acc")
                        for ct in range(HPC):
                            nc.tensor.matmul(
                                py[:],
                                wo_sb[:, ct, et * P:(et + 1) * P],
                                oT[:, ct, jsl],
                                start=(ct == 0), stop=(ct == HPC - 1))
                        nc.vector.tensor_scalar_mul(yt[:, si, :], py[:], 1.0)
                    nc.sync.dma_start(
                        yT[eh * 2 * P:(eh + 1) * 2 * P, jsl]
                        .rearrange("(n p) q -> p n q", p=P),
                        yt[:])
    nc.finalize()
    return nc


def _host_inputs(x, wq, wk, wv, wo):
    """Build per-core input maps (host-side shard + transform)."""
    import ml_dtypes
    bf16 = ml_dtypes.bfloat16
    scale = 1.0 / np.sqrt(np.float32(HD))

    # RoPE tables in [e, s] layout (same for every head)
    inv_freq = 1.0 / (ROPE_THETA ** (np.arange(0, HD, 2, dtype=np.float64) / HD))
    ang = np.arange(S, dtype=np.float64)[None, :] * inv_freq[:, None]  # [64, S]
    cosT = np.repeat(np.cos(ang), 2, axis=0).astype(bf16)  # [128, S]
    sinT = np.repeat(np.sin(ang), 2, axis=0).astype(bf16)

    # signed pair-swap: qrot[2i] = -q[2i+1], qrot[2i+1] = q[2i]
    # matmul computes qrot[m, s] = sum_k rotL[k, m] q[k, s]
    rotL = np.zeros((HD, HD), dtype=np.float32)
    for i in range(HD // 2):
        rotL[2 * i + 1, 2 * i] = -1.0
        rotL[2 * i, 2 * i + 1] = 1.0
    rotL = rotL.astype(bf16)

    r = np.arange(P)[:, None]
    c = np.arange(P)[None, :]
    tri = (c >= r).astype(bf16)  # [128,128] upper-right valid
    trimask = np.concatenate([tri, tri], axis=1)  # [128, 256]

    wq_s = (wq * scale).astype(bf16)
    wk_s = wk.astype(bf16)
    wv_s = wv.astype(bf16)
    wo_s = wo.astype(bf16)
    xTb = [np.ascontiguousarray(x[b].T.astype(bf16)) for b in range(B)]

    in_maps = []
    for cix in range(NCORES):
        b = cix // 4
        g = cix % 4                       # head group (4 heads)
        rows = slice(g * CPC, (g + 1) * CPC)
        blocks = []
        for h in range(HPC):
            hr = slice((g * HPC + h) * HD, (g * HPC + h + 1) * HD)
            blocks.append(wq_s[hr])       # q_h: [128, D]
        for h in range(HPC):
            hr = slice((g * HPC + h) * HD, (g * HPC + h + 1) * HD)
            blocks.append(wk_s[hr])
        blocks.append(wv_s[rows])         # v all 4 heads: [512, D]
        wqkvT = np.ascontiguousarray(
            np.concatenate(blocks, axis=0).T)  # [D, 1536]
        woT = np.ascontiguousarray(wo_s[:, rows].T)  # [512, D]
        in_maps.append({
            "xT": xTb[b],
            "wqkvT": wqkvT,
            "woT": woT,
            "cosT": cosT,
            "sinT": sinT,
            "rotL": rotL,
            "trimask": trimask,
            "ones": np.ones((P, P), dtype=bf16),
        })
    return in_maps


def _get_nc():
    global _nc_cache
    if _nc_cache is None:
        _nc_cache = _build_nc()
    return _nc_cache


def kernel(x, wq, wk, wv, wo, _trace=False):
    global last_exec_time_ns
    nc = _get_nc()
    in_maps = _host_inputs(np.asarray(x, dtype=np.float32),
                           np.asarray(wq, dtype=np.float32),
                           np.asarray(wk, dtype=np.float32),
                           np.asarray(wv, dtype=np.float32),
                           np.asarray(wo, dtype=np.float32))
    res = run_bass_kernel_spmd(nc, in_maps, core_ids=list(range(NCORES)),
                               trace=_trace)
    last_exec_time_ns = res.exec_time_ns
    y = np.zeros((B, S, D), dtype=np.float64)
    for cix in range(NCORES):
        b = cix // 4
        y[b] += res.results[cix]["yT"].T.astype(np.float64)
    return y.astype(np.float32)


# revision 4
# speedup vs baseline: 1.2924x; 1.1370x over previous
"""Multi-head causal self-attention with RoPE on 8 Trainium2 NeuronCores.

Problem: x[2,2048,2048], wq/wk/wv/wo[2048,2048] fp32, 16 heads (hd=128),
interleaved RoPE, causal softmax.

Sharding: (batch, head-group) parallel — each core owns ONE batch and FOUR
heads (cores 0-3 -> b=0 head-groups 0-3, cores 4-7 -> b=1).  wo is
row-sharded; host sums the 4 partial y's per batch.

All data bf16 (same PE rate as f32r, half DMA/SBUF); PSUM stays fp32.

Fused pipeline per core, per s-chunk sc (512 cols):
  - project chunk sc: qT,kT per head [e=128, 512] (+RoPE fused:
    pair-swap matmul + cos/sin on DVE), v natural [s, ch=512];
    all tiles split per-(e,chunk) so dependencies are fine-grained.
  - attention q-block j=sc (kv chunks 0..sc are all projected by now):
    heads in 2 pairs; per kv tile t: scores for both heads into one
    2-bank PSUM tile -> single exp (ACT) with a 3D AP covering both
    heads' valid staircase regions -> attnV + ones-rowsum matmuls of the
    PREVIOUS tile run while exp(t) is on ACT (1-deep pend queue that
    also crosses pair boundaries).
  - output projection for block j; y DMA'd out per 256-row slab (bf16).
"""

import os
import sys

for _p in ("/opt/trn_rl_repo", "/root/.axon_site/_ro/trn_rl_repo"):
    if os.path.isdir(_p) and _p not in sys.path:
        sys.path.append(_p)

import numpy as np

import concourse.bacc as bacc
import concourse.mybir as mybir
import concourse.tile as tile
from concourse.alu_op_type import AluOpType
from concourse.bass_utils import run_bass_kernel_spmd

F32 = mybir.dt.float32
BF16 = mybir.dt.bfloat16

B, S, D = 2, 2048, 2048
H, HD = 16, 128
NCORES = 8
HPC = 4                      # heads per core
CPC = HPC * HD               # channels per core = 512
P = 128
SC = 512                     # s-chunk for projections / q-block for attention
NSC = S // SC                # 4
NDT = D // P                 # 16 contraction tiles
NG = 4                       # x-tile DMA group: d-tiles per DMA
WCOLS = 8 * P + CPC          # 1536: q h0..3, k h0..3, v (512)
ROPE_THETA = 10000.0

Exp = mybir.ActivationFunctionType.Exp

last_exec_time_ns = None
_nc_cache = None


def _build_nc():
    nc = bacc.Bacc("TRN2", target_bir_lowering=False, debug=False)

    xT = nc.dram_tensor("xT", [D, S], BF16, kind="ExternalInput")
    wqkvT = nc.dram_tensor("wqkvT", [D, WCOLS], BF16, kind="ExternalInput")
    woT = nc.dram_tensor("woT", [CPC, D], BF16, kind="ExternalInput")
    cosT = nc.dram_tensor("cosT", [HD, S], BF16, kind="ExternalInput")
    sinT = nc.dram_tensor("sinT", [HD, S], BF16, kind="ExternalInput")
    rotL = nc.dram_tensor("rotL", [HD, HD], BF16, kind="ExternalInput")
    trimask = nc.dram_tensor("trimask", [P, 2 * P], BF16, kind="ExternalInput")
    ones = nc.dram_tensor("ones", [P, P], BF16, kind="ExternalInput")
    yT = nc.dram_tensor("yT", [D, S], BF16, kind="ExternalOutput")

    xTr = xT.rearrange("(o p) s -> p o s", p=P)
    wqr = wqkvT.rearrange("(o p) e -> p o e", p=P)

    with tile.TileContext(nc) as tc:
        with tc.tile_pool(name="const", bufs=1) as constp, \
             tc.tile_pool(name="xp", bufs=6) as xp, \
             tc.tile_pool(name="qk", bufs=1) as qkp, \
             tc.tile_pool(name="vp", bufs=1) as vp, \
             tc.tile_pool(name="op", bufs=2) as op_, \
             tc.tile_pool(name="attn", bufs=4) as attnp, \
             tc.tile_pool(name="tmp", bufs=2) as tmpp, \
             tc.tile_pool(name="rc", bufs=4) as rcp, \
             tc.tile_pool(name="yt", bufs=2) as ytp, \
             tc.tile_pool(name="ps", bufs=2, space="PSUM") as psp, \
             tc.tile_pool(name="acc", bufs=4, space="PSUM") as accp:

            # ---- constants: wq per d-tile (fine-grained deps); small RoPE
            #      tables early so the first chunk's RoPE never stalls ----
            wq_t = []
            for dt in range(NDT):
                w = constp.tile([P, WCOLS], BF16, name=f"wq{dt}")
                nc.sync.dma_start(w[:], wqr[:, dt, :])
                wq_t.append(w)
                if dt == 0:
                    rot_sb = constp.tile([P, P], BF16)
                    cos_sb = constp.tile([P, S], BF16)
                    sin_sb = constp.tile([P, S], BF16)
                    mask_sb = constp.tile([P, 2, P], BF16)
                    ones_sb = constp.tile([P, P], BF16)
                    nc.sync.dma_start(rot_sb[:], rotL[:])
                    nc.sync.dma_start(cos_sb[:], cosT[:])
                    nc.sync.dma_start(sin_sb[:], sinT[:])
                    nc.sync.dma_start(
                        mask_sb[:], trimask.rearrange("p (n q) -> p n q", n=2))
                    nc.sync.dma_start(ones_sb[:], ones[:])
            wo_sb = constp.tile([P, HPC, D], BF16)
            nc.sync.dma_start(wo_sb[:], woT.rearrange("(o p) e -> p o e", p=P))

            # qkc[e][sc]: [128, 512] per (channel-tile, s-chunk); e 0-3 = q
            # heads, 4-7 = k heads.  v_c[sc]: [s=128, ss, ch=512] per chunk.
            qkc = [[qkp.tile([P, SC], BF16, tag=f"qk{e}_{c}", name=f"qk{e}_{c}")
                    for c in range(NSC)] for e in range(8)]
            v_c = [vp.tile([P, SC // P, CPC], BF16, tag=f"v{c}", name=f"v{c}")
                   for c in range(NSC)]

            for sc in range(NSC):
                # ---------------- projection of chunk sc ----------------
                xts = []
                for g in range(NDT // NG):
                    xt = xp.tile([P, NG, SC], BF16, tag="xt")
                    nc.gpsimd.dma_start(
                        xt[:], xTr[:, g * NG:(g + 1) * NG,
                                   sc * SC:(sc + 1) * SC])
                    xts.append(xt)
                for e in range(8):
                    pq = accp.tile([P, SC], F32, tag="acc", name=f"pq{e}_{sc}")
                    for dt in range(NDT):
                        nc.tensor.matmul(pq[:],
                                         wq_t[dt][:, e * P:(e + 1) * P],
                                         xts[dt // NG][:, dt % NG, :],
                                         start=(dt == 0), stop=(dt == NDT - 1))
                    q = qkc[e][sc]
                    nc.vector.tensor_scalar_mul(q[:], pq[:], 1.0)
                    pr = psp.tile([P, 2, SC], F32, tag="ps", name=f"pr{e}_{sc}")
                    nc.tensor.matmul(pr[:, 0, :], rot_sb[:], q[:],
                                     start=True, stop=True)
                    tmp = tmpp.tile([P, SC], BF16, tag="ropetmp")
                    sl = slice(sc * SC, (sc + 1) * SC)
                    nc.vector.tensor_tensor(tmp[:], pr[:, 0, :], sin_sb[:, sl],
                                            AluOpType.mult)
                    nc.vector.tensor_tensor(q[:], q[:], cos_sb[:, sl],
                                            AluOpType.mult)
                    nc.vector.tensor_tensor(q[:], q[:], tmp[:], AluOpType.add)
                for ss in range(SC // P):
                    pv = accp.tile([P, SC], F32, tag="acc", name=f"pv{ss}_{sc}")
                    for dt in range(NDT):
                        nc.tensor.matmul(pv[:],
                                         xts[dt // NG][:, dt % NG,
                                                       ss * P:(ss + 1) * P],
                                         wq_t[dt][:, 8 * P:],
                                         start=(dt == 0), stop=(dt == NDT - 1))
                    nc.vector.tensor_scalar_mul(v_c[sc][:, ss, :], pv[:], 1.0)

                # ---------------- attention q-block j = sc ----------------
                j = sc
                n_kv = (SC // P) * (j + 1)
                oT = op_.tile([P, HPC, SC], BF16, tag="o", name=f"oT{j}")
                pend = []  # one in-flight (pair_state, attn, t, dlt)

                def drain_one():
                    st, attn, t, dlt = pend.pop(0)
                    first, last = (t == 0), (t == n_kv - 1)
                    for i in range(2):
                        nc.tensor.matmul(st["po"][i][:, dlt:],
                                         v_c[t // 4][:, t % 4,
                                                     st["h"][i] * HD:
                                                     (st["h"][i] + 1) * HD],
                                         attn[:, i, dlt:],
                                         start=first, stop=last,
                                         skip_group_check=True)
                        nc.tensor.matmul(st["prs"][i][:, dlt:], ones_sb[:],
                                         attn[:, i, dlt:],
                                         start=first, stop=last,
                                         skip_group_check=True)
                    if last:  # this pair is done: normalize its output
                        for i in range(2):
                            h = st["h"][i]
                            recip = rcp.tile([P, SC], F32, tag="recip")
                            nc.vector.reciprocal_approx_fast(recip[:],
                                                             st["prs"][i][:])
                            nc.vector.tensor_tensor(oT[:, h, :],
                                                    st["po"][i][:], recip[:],
                                                    AluOpType.mult)

                for pair in range(2):
                    hA, hB = 2 * pair, 2 * pair + 1
                    st = {
                        "h": (hA, hB),
                        "po": [accp.tile([P, SC], F32, tag="acc",
                                         name=f"po{j}_{pair}_{i}")
                               for i in range(2)],
                        "prs": [accp.tile([P, SC], F32, tag="acc",
                                          name=f"prs{j}_{pair}_{i}")
                                for i in range(2)],
                    }
                    for t in range(n_kv):
                        dp = t - (SC // P) * j
                        dlt = max(dp, 0) * P  # first valid column
                        vsl = slice(dlt, SC)
                        tsl = slice((t % 4) * P, (t % 4 + 1) * P)
                        psc = psp.tile([P, 2, SC], F32, tag="ps",
                                       name=f"psc{j}_{pair}_{t}")
                        for i, h in enumerate((hA, hB)):
                            nc.tensor.matmul(psc[:, i, dlt:],
                                             qkc[4 + h][t // 4][:, tsl],
                                             qkc[h][j][:, vsl],
                                             start=True, stop=True)
                        attn = attnp.tile([P, 2, SC], BF16, tag="attn")
                        nc.scalar.activation(attn[:, :, dlt:], psc[:, :, dlt:],
                                             Exp, bias=0.0, scale=1.0)
                        if dp >= 0:  # triangle mask on the 128-col band
                            nc.vector.tensor_tensor(
                                attn[:, :, dlt:dlt + P],
                                attn[:, :, dlt:dlt + P],
                                mask_sb[:], AluOpType.mult)
                        pend.append((st, attn, t, dlt))
                        if len(pend) > 1:
                            drain_one()
                while pend:
                    drain_one()

                # ---------------- output projection for block j ----------------
                jsl = slice(j * SC, (j + 1) * SC)
                for eh in range(NDT // 2):
                    yt = ytp.tile([P, 2, SC], BF16, tag="yt")
                    for si in range(2):
                        et = eh * 2 + si
                        py = accp.tile([P, SC], F32, tag="acc",
                                       name=f"py{j}_{et}")
                        for ct in range(HPC):
                            nc.tensor.matmul(
                                py[:],
                                wo_sb[:, ct, et * P:(et + 1) * P],
                                oT[:, ct, :],
                                start=(ct == 0), stop=(ct == HPC - 1))
                        nc.vector.tensor_scalar_mul(yt[:, si, :], py[:], 1.0)
                    nc.gpsimd.dma_start(
                        yT[eh * 2 * P:(eh + 1) * 2 * P, jsl]
                        .rearrange("(n p) q -> p n q", p=P),
                        yt[:])
    nc.finalize()
    return nc


def _host_inputs(x, wq, wk, wv, wo):
    """Build per-core input maps (host-side shard + transform)."""
    import ml_dtypes
    bf16 = ml_dtypes.bfloat16
    scale = 1.0 / np.sqrt(np.float32(HD))

    # RoPE tables in [e, s] layout (same for every head)
    inv_freq = 1.0 / (ROPE_THETA ** (np.arange(0, HD, 2, dtype=np.float64) / HD))
    ang = np.arange(S, dtype=np.float64)[None, :] * inv_freq[:, None]  # [64, S]
    cosT = np.repeat(np.cos(ang), 2, axis=0).astype(bf16)  # [128, S]
    sinT = np.repeat(np.sin(ang), 2, axis=0).astype(bf16)

    # signed pair-swap: qrot[2i] = -q[2i+1], qrot[2i+1] = q[2i]
    # matmul computes qrot[m, s] = sum_k rotL[k, m] q[k, s]
    rotL = np.zeros((HD, HD), dtype=np.float32)
    for i in range(HD // 2):
        rotL[2 * i + 1, 2 * i] = -1.0
        rotL[2 * i, 2 * i + 1] = 1.0
    rotL = rotL.astype(bf16)

    r = np.arange(P)[:, None]
    c = np.arange(P)[None, :]
    tri = (c >= r).astype(bf16)  # [128,128] upper-right valid
    trimask = np.concatenate([tri, tri], axis=1)  # [128, 256]

    wq_s = (wq * scale).astype(bf16)
    wk_s = wk.astype(bf16)
    wv_s = wv.astype(bf16)
    wo_s = wo.astype(bf16)
    xTb = [np.ascontiguousarray(x[b].T.astype(bf16)) for b in range(B)]

    in_maps = []
    for cix in range(NCORES):
        b = cix // 4
        g = cix % 4                       # head group (4 heads)
        rows = slice(g * CPC, (g + 1) * CPC)
        blocks = []
        for h in range(HPC):
            hr = slice((g * HPC + h) * HD, (g * HPC + h + 1) * HD)
            blocks.append(wq_s[hr])       # q_h: [128, D]
        for h in range(HPC):
            hr = slice((g * HPC + h) * HD, (g * HPC + h + 1) * HD)
            blocks.append(wk_s[hr])
        blocks.append(wv_s[rows])         # v all 4 heads: [512, D]
        wqkvT = np.ascontiguousarray(
            np.concatenate(blocks, axis=0).T)  # [D, 1536]
        woT = np.ascontiguousarray(wo_s[:, rows].T)  # [512, D]
        in_maps.append({
            "xT": xTb[b],
            "wqkvT": wqkvT,
            "woT": woT,
            "cosT": cosT,
            "sinT": sinT,
            "rotL": rotL,
            "trimask": trimask,
            "ones": np.ones((P, P), dtype=bf16),
        })
    return in_maps


def _get_nc():
    global _nc_cache
    if _nc_cache is None:
        _nc_cache = _build_nc()
    return _nc_cache


def kernel(x, wq, wk, wv, wo, _trace=False):
    global last_exec_time_ns
    nc = _get_nc()
    in_maps = _host_inputs(np.asarray(x, dtype=np.float32),
                           np.asarray(wq, dtype=np.float32),
                           np.asarray(wk, dtype=np.float32),
                           np.asarray(wv, dtype=np.float32),
                           np.asarray(wo, dtype=np.float32))
    res = run_bass_kernel_spmd(nc, in_maps, core_ids=list(range(NCORES)),
                               trace=_trace)
    last_exec_time_ns = res.exec_time_ns
    y = np.zeros((B, S, D), dtype=np.float64)
    for cix in range(NCORES):
        b = cix // 4
        y[b] += res.results[cix]["yT"].T.astype(np.float64)
    return y.astype(np.float32)


# revision 5
# speedup vs baseline: 1.3195x; 1.0209x over previous
"""Multi-head causal self-attention with RoPE on 8 Trainium2 NeuronCores.

Problem: x[2,2048,2048], wq/wk/wv/wo[2048,2048] fp32, 16 heads (hd=128),
interleaved RoPE, causal softmax.

Sharding: (batch, head-group) parallel — each core owns ONE batch and FOUR
heads (cores 0-3 -> b=0 head-groups 0-3, cores 4-7 -> b=1).  wo is
row-sharded; host sums the 4 partial y's per batch.

All data bf16 (1 col/cycle PE rate, half DMA/SBUF); PSUM stays fp32.

Fused pipeline per core, per s-chunk sc (512 cols):
  - project chunk sc: qT,kT per head [e=128, 512] (+RoPE fused), v natural.
  - attention q-block j=sc: heads in 2 pairs; per kv tile t: scores for
    both heads into one 2-bank PSUM tile -> single exp (ACT, 3D AP over
    both heads' valid staircase regions) -> mask -> DVE-accumulate into a
    per-pair f32r rowsum-accumulator; attnV matmuls of the PREVIOUS tile
    run while exp(t) is on ACT.  Softmax denominator = ONE ones-matmul on
    the accumulator per (pair, head) instead of per kv tile (keeps the
    partition-reduce on the PE but at 1/16 the columns).
  - output projection for block j is DEFERRED: its matmuls are emitted
    interleaved into attention block j+1's t-loop, filling the PE slots
    that the ACT-bound exp stream would otherwise leave idle.

DMA: weight d-tiles split across both rings (sync: even, gpsimd: odd)
interleaved with the first x chunk so the first projection chain is never
starved; y leaves per 128-row slab on the sync ring (idle late).
"""

import os
import sys

for _p in ("/opt/trn_rl_repo", "/root/.axon_site/_ro/trn_rl_repo"):
    if os.path.isdir(_p) and _p not in sys.path:
        sys.path.append(_p)

import numpy as np

import concourse.bacc as bacc
import concourse.mybir as mybir
import concourse.tile as tile
from concourse.alu_op_type import AluOpType
from concourse.bass_utils import run_bass_kernel_spmd

F32 = mybir.dt.float32
F32R = mybir.dt.float32r
BF16 = mybir.dt.bfloat16

B, S, D = 2, 2048, 2048
H, HD = 16, 128
NCORES = 8
HPC = 4                      # heads per core
CPC = HPC * HD               # channels per core = 512
P = 128
SC = 512                     # s-chunk for projections / q-block for attention
NSC = S // SC                # 4
NDT = D // P                 # 16 contraction tiles
NG = 4                       # x-tile DMA group: d-tiles per DMA
WCOLS = 8 * P + CPC          # 1536: q h0..3, k h0..3, v (512)
ROPE_THETA = 10000.0

Exp = mybir.ActivationFunctionType.Exp

last_exec_time_ns = None
_nc_cache = None


def _build_nc():
    nc = bacc.Bacc("TRN2", target_bir_lowering=False, debug=False)

    xT = nc.dram_tensor("xT", [D, S], BF16, kind="ExternalInput")
    wqkvT = nc.dram_tensor("wqkvT", [D, WCOLS], BF16, kind="ExternalInput")
    woT = nc.dram_tensor("woT", [CPC, D], BF16, kind="ExternalInput")
    cosT = nc.dram_tensor("cosT", [HD, S], BF16, kind="ExternalInput")
    sinT = nc.dram_tensor("sinT", [HD, S], BF16, kind="ExternalInput")
    rotL = nc.dram_tensor("rotL", [HD, HD], BF16, kind="ExternalInput")
    trimask = nc.dram_tensor("trimask", [P, 2 * P], BF16, kind="ExternalInput")
    ones = nc.dram_tensor("ones", [P, P], F32R, kind="ExternalInput")
    yT = nc.dram_tensor("yT", [D, S], BF16, kind="ExternalOutput")

    xTr = xT.rearrange("(o p) s -> p o s", p=P)
    wqr = wqkvT.rearrange("(o p) e -> p o e", p=P)

    with tile.TileContext(nc) as tc:
        with tc.tile_pool(name="const", bufs=1) as constp, \
             tc.tile_pool(name="xp", bufs=6) as xp, \
             tc.tile_pool(name="qk", bufs=1) as qkp, \
             tc.tile_pool(name="vp", bufs=1) as vp, \
             tc.tile_pool(name="op", bufs=2) as op_, \
             tc.tile_pool(name="attn", bufs=4) as attnp, \
             tc.tile_pool(name="acs", bufs=2) as accsp, \
             tc.tile_pool(name="tmp", bufs=2) as tmpp, \
             tc.tile_pool(name="rc", bufs=4) as rcp, \
             tc.tile_pool(name="yt", bufs=4) as ytp, \
             tc.tile_pool(name="ps", bufs=2, space="PSUM") as psp, \
             tc.tile_pool(name="po", bufs=1, space="PSUM") as pop, \
             tc.tile_pool(name="acc", bufs=2, space="PSUM") as accp:

            # ---- weights: per d-tile, split across BOTH DMA rings so the
            #      first accumulation chain is fed as fast as possible ----
            wq_t = [constp.tile([P, WCOLS], BF16, name=f"wq{dt}")
                    for dt in range(NDT)]
            xts0 = []   # first x chunk, interleaved with odd weight tiles
            for g in range(NDT // NG):
                xt = xp.tile([P, NG, SC], BF16, tag="xt", name=f"xt0_{g}")
                xts0.append(xt)
            nc.gpsimd.dma_start(xts0[0][:], xTr[:, 0:NG, 0:SC])
            nc.sync.dma_start(wq_t[0][:], wqr[:, 0, :])
            for dt in range(1, NDT, 2):
                nc.gpsimd.dma_start(wq_t[dt][:], wqr[:, dt, :])
            for dt in range(2, NDT, 2):
                nc.sync.dma_start(wq_t[dt][:], wqr[:, dt, :])
            for g in range(1, NDT // NG):
                nc.gpsimd.dma_start(
                    xts0[g][:], xTr[:, g * NG:(g + 1) * NG, 0:SC])
            rot_sb = constp.tile([P, P], BF16)
            cos_sb = constp.tile([P, S], BF16)
            sin_sb = constp.tile([P, S], BF16)
            mask_sb = constp.tile([P, 2, P], BF16)
            ones_sb = constp.tile([P, P], F32R)
            nc.sync.dma_start(rot_sb[:], rotL[:])
            nc.sync.dma_start(cos_sb[:], cosT[:])
            nc.sync.dma_start(sin_sb[:], sinT[:])
            nc.sync.dma_start(
                mask_sb[:], trimask.rearrange("p (n q) -> p n q", n=2))
            nc.sync.dma_start(ones_sb[:], ones[:])
            wo_sb = constp.tile([P, HPC, D], BF16)
            nc.sync.dma_start(wo_sb[:], woT.rearrange("(o p) e -> p o e", p=P))

            qkc = [[qkp.tile([P, SC], BF16, tag=f"qk{e}_{c}", name=f"qk{e}_{c}")
                    for c in range(NSC)] for e in range(8)]
            v_c = [vp.tile([P, SC // P, CPC], BF16, tag=f"v{c}", name=f"v{c}")
                   for c in range(NSC)]

            def proj_chunk(sc, xts):
                for e in range(8):
                    pq = accp.tile([P, SC], F32, tag="acc", name=f"pq{e}_{sc}")
                    for dt in range(NDT):
                        nc.tensor.matmul(pq[:],
                                         wq_t[dt][:, e * P:(e + 1) * P],
                                         xts[dt // NG][:, dt % NG, :],
                                         start=(dt == 0), stop=(dt == NDT - 1))
                    q = qkc[e][sc]
                    nc.vector.tensor_scalar_mul(q[:], pq[:], 1.0)
                    pr = psp.tile([P, 2, SC], F32, tag="ps", name=f"pr{e}_{sc}")
                    nc.tensor.matmul(pr[:, 0, :], rot_sb[:], q[:],
                                     start=True, stop=True)
                    tmp = tmpp.tile([P, SC], BF16, tag="ropetmp")
                    sl = slice(sc * SC, (sc + 1) * SC)
                    nc.vector.tensor_tensor(tmp[:], pr[:, 0, :], sin_sb[:, sl],
                                            AluOpType.mult)
                    nc.vector.tensor_tensor(q[:], q[:], cos_sb[:, sl],
                                            AluOpType.mult)
                    nc.vector.tensor_tensor(q[:], q[:], tmp[:], AluOpType.add)
                for ss in range(SC // P):
                    pv = accp.tile([P, SC], F32, tag="acc", name=f"pv{ss}_{sc}")
                    for dt in range(NDT):
                        nc.tensor.matmul(pv[:],
                                         xts[dt // NG][:, dt % NG,
                                                       ss * P:(ss + 1) * P],
                                         wq_t[dt][:, 8 * P:],
                                         start=(dt == 0), stop=(dt == NDT - 1))
                    nc.vector.tensor_scalar_mul(v_c[sc][:, ss, :], pv[:], 1.0)

            def attn_block(j, deferred):
                """Attention q-block j; emits `deferred` closures (the
                previous block's output projection) into PE idle slots.
                Returns the closures for THIS block's output projection."""
                n_kv = (SC // P) * (j + 1)
                oT = op_.tile([P, HPC, SC], BF16, tag="o", name=f"oT{j}")
                pend = []

                def drain_one():
                    st, attn, t, dlt = pend.pop(0)
                    first, last = (t == 0), (t == n_kv - 1)
                    for i in range(2):
                        nc.tensor.matmul(st["po"][:, i, dlt:],
                                         v_c[t // 4][:, t % 4,
                                                     st["h"][i] * HD:
                                                     (st["h"][i] + 1) * HD],
                                         attn[:, i, dlt:],
                                         start=first, stop=last,
                                         skip_group_check=True)
                    if last:  # pair done: denominator matmul + normalize
                        for i in range(2):
                            h = st["h"][i]
                            prs = accp.tile([P, SC], F32, tag="acc",
                                            name=f"prs{j}_{h}")
                            nc.tensor.matmul(prs[:], ones_sb[:],
                                             st["acc"][:, i, :],
                                             start=True, stop=True)
                            recip = rcp.tile([P, SC], F32, tag="recip")
                            nc.vector.reciprocal_approx_fast(recip[:], prs[:])
                            nc.vector.tensor_tensor(oT[:, h, :],
                                                    st["po"][:, i, :],
                                                    recip[:], AluOpType.mult)

                for pair in range(2):
                    hA, hB = 2 * pair, 2 * pair + 1
                    st = {
                        "h": (hA, hB),
                        "po": pop.tile([P, 2, SC], F32, tag="po",
                                       name=f"po{j}_{pair}"),
                        "acc": accsp.tile([P, 2, SC], F32R, tag="acs",
                                          name=f"acs{j}_{pair}"),
                    }
                    for t in range(n_kv):
                        dp = t - (SC // P) * j
                        dlt = max(dp, 0) * P  # first valid column
                        psc = psp.tile([P, 2, SC], F32, tag="ps",
                                       name=f"psc{j}_{pair}_{t}")
                        for i, h in enumerate((hA, hB)):
                            nc.tensor.matmul(psc[:, i, dlt:],
                                             qkc[4 + h][t // 4][:, (t % 4) * P:
                                                                (t % 4 + 1) * P],
                                             qkc[h][j][:, dlt:],
                                             start=True, stop=True)
                        attn = attnp.tile([P, 2, SC], BF16, tag="attn")
                        nc.scalar.activation(attn[:, :, dlt:], psc[:, :, dlt:],
                                             Exp, bias=0.0, scale=1.0)
                        if dp >= 0:  # triangle mask on the 128-col band
                            nc.vector.tensor_tensor(
                                attn[:, :, dlt:dlt + P],
                                attn[:, :, dlt:dlt + P],
                                mask_sb[:], AluOpType.mult)
                        if t == 0:  # rowsum accumulator init / accumulate
                            nc.vector.tensor_scalar_mul(st["acc"][:],
                                                        attn[:], 1.0)
                        else:
                            nc.vector.tensor_tensor(st["acc"][:, :, dlt:],
                                                    st["acc"][:, :, dlt:],
                                                    attn[:, :, dlt:],
                                                    AluOpType.add)
                        pend.append((st, attn, t, dlt))
                        if len(pend) > 1:
                            drain_one()
                        if deferred:
                            deferred.pop(0)()
                while pend:
                    drain_one()
                while deferred:
                    deferred.pop(0)()

                jsl = slice(j * SC, (j + 1) * SC)

                def mk_closure(et):
                    def emit():
                        py = accp.tile([P, SC], F32, tag="acc",
                                       name=f"py{j}_{et}")
                        for ct in range(HPC):
                            nc.tensor.matmul(
                                py[:],
                                wo_sb[:, ct, et * P:(et + 1) * P],
                                oT[:, ct, :],
                                start=(ct == 0), stop=(ct == HPC - 1))
                        yt = ytp.tile([P, SC], BF16, tag="yt")
                        nc.vector.tensor_scalar_mul(yt[:], py[:], 1.0)
                        nc.sync.dma_start(
                            yT[et * P:(et + 1) * P, jsl], yt[:])
                    return emit

                return [mk_closure(et) for et in range(NDT)]

            deferred = []
            for sc in range(NSC):
                if sc == 0:
                    xts = xts0
                else:
                    xts = []
                    for g in range(NDT // NG):
                        xt = xp.tile([P, NG, SC], BF16, tag="xt")
                        nc.gpsimd.dma_start(
                            xt[:], xTr[:, g * NG:(g + 1) * NG,
                                       sc * SC:(sc + 1) * SC])
                        xts.append(xt)
                proj_chunk(sc, xts)
                deferred = attn_block(sc, deferred)
            for cl in deferred:  # final block's output projection
                cl()
    nc.finalize()
    return nc


def _host_inputs(x, wq, wk, wv, wo):
    """Build per-core input maps (host-side shard + transform)."""
    import ml_dtypes
    bf16 = ml_dtypes.bfloat16
    scale = 1.0 / np.sqrt(np.float32(HD))

    # RoPE tables in [e, s] layout (same for every head)
    inv_freq = 1.0 / (ROPE_THETA ** (np.arange(0, HD, 2, dtype=np.float64) / HD))
    ang = np.arange(S, dtype=np.float64)[None, :] * inv_freq[:, None]  # [64, S]
    cosT = np.repeat(np.cos(ang), 2, axis=0).astype(bf16)  # [128, S]
    sinT = np.repeat(np.sin(ang), 2, axis=0).astype(bf16)

    # signed pair-swap: qrot[2i] = -q[2i+1], qrot[2i+1] = q[2i]
    # matmul computes qrot[m, s] = sum_k rotL[k, m] q[k, s]
    rotL = np.zeros((HD, HD), dtype=np.float32)
    for i in range(HD // 2):
        rotL[2 * i + 1, 2 * i] = -1.0
        rotL[2 * i, 2 * i + 1] = 1.0
    rotL = rotL.astype(bf16)

    r = np.arange(P)[:, None]
    c = np.arange(P)[None, :]
    tri = (c >= r).astype(bf16)  # [128,128] upper-right valid
    trimask = np.concatenate([tri, tri], axis=1)  # [128, 256]

    wq_s = (wq * scale).astype(bf16)
    wk_s = wk.astype(bf16)
    wv_s = wv.astype(bf16)
    wo_s = wo.astype(bf16)
    xTb = [np.ascontiguousarray(x[b].T.astype(bf16)) for b in range(B)]

    in_maps = []
    for cix in range(NCORES):
        b = cix // 4
        g = cix % 4                       # head group (4 heads)
        rows = slice(g * CPC, (g + 1) * CPC)
        blocks = []
        for h in range(HPC):
            hr = slice((g * HPC + h) * HD, (g * HPC + h + 1) * HD)
            blocks.append(wq_s[hr])       # q_h: [128, D]
        for h in range(HPC):
            hr = slice((g * HPC + h) * HD, (g * HPC + h + 1) * HD)
            blocks.append(wk_s[hr])
        blocks.append(wv_s[rows])         # v all 4 heads: [512, D]
        wqkvT = np.ascontiguousarray(
            np.concatenate(blocks, axis=0).T)  # [D, 1536]
        woT = np.ascontiguousarray(wo_s[:, rows].T)  # [512, D]
        in_maps.append({
            "xT": xTb[b],
            "wqkvT": wqkvT,
            "woT": woT,
            "cosT": cosT,
            "sinT": sinT,
            "rotL": rotL,
            "trimask": trimask,
            "ones": np.ones((P, P), dtype=np.float32),
        })
    return in_maps


def _get_nc():
    global _nc_cache
    if _nc_cache is None:
        _nc_cache = _build_nc()
    return _nc_cache


def kernel(x, wq, wk, wv, wo, _trace=False):
    global last_exec_time_ns
    nc = _get_nc()
    in_maps = _host_inputs(np.asarray(x, dtype=np.float32),
                           np.asarray(wq, dtype=np.float32),
                           np.asarray(wk, dtype=np.float32),
                           np.asarray(wv, dtype=np.float32),
                           np.asarray(wo, dtype=np.float32))
    res = run_bass_kernel_spmd(nc, in_maps, core_ids=list(range(NCORES)),
                               trace=_trace)
    last_exec_time_ns = res.exec_time_ns
    y = np.zeros((B, S, D), dtype=np.float64)
    for cix in range(NCORES):
        b = cix // 4
        y[b] += res.results[cix]["yT"].T.astype(np.float64)
    return y.astype(np.float32)
